# revision 60
# baseline (speedup 1.0000x reference)
"""Multi-head attention (RoPE, causal) Trainium2 Bass kernel.

Sharding (8 cores): data-parallel over batch (4) x tensor-parallel over
heads (16 -> 2 groups of 8).  Core c handles batch c//2 and head group
c%2.  Attention is fully head-local; the out-projection partial sums of
the two head groups of each batch are added on the host.

v3 design (sizes hardcoded for b=4, n=2048, hidden=1024, h=16, d=64):
  - fp16 end to end (PSUM accumulation stays f32).  The softmax scale
    1/8 and a constant -3 shift (so p fits fp16 comfortably) fold into
    the exp activation; the shift cancels in the normalization.
  - Padding folds into the exp bias (per-k-partition scalar), so the
    denominator ones-column in v is constant and v needs no scaling.
  - Both heads of a pair write score PSUM banks side by side in one
    [128,1024] 2-bank tile -> a single exp instruction per (pair,kc).
  - Causal masking multiplies the exp output (diagonal blocks only).
  - AV lags the exp by two k-blocks so the PE never waits on ScalarE.
  - Software pipelining: the projection matmuls of strip ic+1 and the
    out-projection of strip ic-1 are chopped into single-matmul units
    and pumped into the attention instruction stream, so the PE stays
    busy through the exp-bound attention inner loop.
  - RoPE: sin/cos are equal within each rotation pair, so the sin
    multiply happens before the pair-permutation matmul (whose output
    accumulates in the projection's own PSUM tile); all elementwise
    work is fp16 on DVE.
  - Startup DMAs are ordered by first use across two hardware queues
    (SP: x strips, Activation: weights/tables), sized to amortize the
    per-descriptor DGE overhead.
"""

import numpy as np

import concourse.bass as bass
import concourse.mybir as mybir
from concourse import bacc
from concourse.tile import TileContext
from concourse.bass_utils import run_bass_kernel_spmd

# ---------------------------------------------------------------- constants
B, N, HID = 4, 2048, 1024
H = 16
D = HID // H                     # 64
NCORES = 8
GROUPS = NCORES // B             # 2 head groups
HPG = H // GROUPS                # 8 heads per core
HD = HPG * D                     # 512 local head dims
PAIRS = HPG // 2                 # 4 head pairs per core
ROPE_THETA = 10000.0
SCALE = 0.125                    # 1/sqrt(d)
ESHIFT = -3.0                    # exp bias shift, cancels in normalization

P = 128
CC = HID // P                    # 8 contraction chunks for projections
ICH = 512                        # projection i-chunk (moving free dim)
QCH = 512                        # attention q-chunk
KCH = 128                        # attention k-chunk
NQC = N // QCH                   # 4
NKC = N // KCH                   # 16
NSTRIP = N // ICH                # 4

F32 = mybir.dt.float32
F32R = mybir.dt.float32r
F16 = mybir.dt.float16

_NC_CACHE = {}
REPEAT = 1
LAG = 4
TWOBANK = True    # single [128,1024] exp spanning 2 PSUM banks per (pair,kc)


# ---------------------------------------------------------------- host prep
def _allow_matrix(is_causal, start_pos):
    i = np.arange(N)[:, None]    # query index
    j = np.arange(N)[None, :]    # key index
    if is_causal:
        return (j < start_pos) | ((i >= start_pos) & (i >= j))
    return np.ones((N, N), dtype=bool)


def _block_plan(is_causal, start_pos):
    """Classify each (qc, kc) score block; return plan + mask table.

    plan[(qc, kc)] is 'skip', 'full', or ('partial', variant_idx).
    The mask table holds, per variant, a [128, 2*QCH] 0/1 fp16 block
    (the same [128, QCH] mask duplicated for the two heads of a pair).
    """
    allow = _allow_matrix(is_causal, start_pos)
    plan = {}
    variants = []
    vkeys = {}
    for qc in range(NQC):
        for kc in range(NKC):
            blk = allow[qc * QCH:(qc + 1) * QCH, kc * KCH:(kc + 1) * KCH]
            if not blk.any():
                plan[(qc, kc)] = ("skip", None)
                continue
            if blk.all():
                plan[(qc, kc)] = ("full", None)
                continue
            bT = blk.T               # [128 k, 512 q]
            start = np.argmax(bT, axis=1)
            for r in range(KCH):
                if not bT[r].any():
                    raise NotImplementedError("empty k-row in partial block")
                s = start[r]
                if not bT[r, s:].all() or bT[r, :s].any():
                    raise NotImplementedError("non-suffix mask row")
            key = start.tobytes()
            if key not in vkeys:
                vkeys[key] = len(variants)
                variants.append(start.astype(np.float32))
            plan[(qc, kc)] = ("partial", vkeys[key])
    if not variants:
        variants.append(np.zeros(KCH, dtype=np.float32))
    q = np.arange(QCH)[None, :]
    blocks = []
    for v in variants:
        m = (q >= v[:, None]).astype(np.float16)      # [128, QCH]
        blocks.append(np.concatenate([m, m], axis=1))  # [128, 2*QCH]
    masks = np.concatenate(blocks, axis=1)             # [128, V*2*QCH]
    return plan, masks


def _rope_tables():
    inv_freq = 1.0 / (ROPE_THETA ** (np.arange(0, D, 2, dtype=np.float64) / D))
    t = np.arange(N, dtype=np.float64)
    freqs = t[:, None] * inv_freq[None, :]        # [N, 32]
    freqs = np.repeat(freqs, 2, axis=1)           # [N, 64]
    cos = np.cos(freqs).T.astype(np.float16)      # [64, N]
    sin = np.sin(freqs).T.astype(np.float16)
    cos2 = np.concatenate([cos, cos], axis=0)     # [128, N]
    sin2 = np.concatenate([sin, sin], axis=0)
    return np.ascontiguousarray(cos2), np.ascontiguousarray(sin2)


def _perm_matrix():
    # rot = PM @ q  with rot[2r] = -q[2r+1], rot[2r+1] = q[2r].
    # matmul computes lhsT.T @ rhs, so pass PM.T.
    pm = np.zeros((P, P), dtype=np.float16)
    for r in range(P // 2):
        pm[2 * r, 2 * r + 1] = -1.0
        pm[2 * r + 1, 2 * r] = 1.0
    return np.ascontiguousarray(pm.T)


# ---------------------------------------------------------------- device IR
def _build_nc(is_causal, start_pos):
    plan, masks_np = _block_plan(is_causal, start_pos)
    nvar = masks_np.shape[1] // (2 * QCH)
    streaming = bool(is_causal)

    nc = bacc.Bacc("TRN2", target_bir_lowering=False, debug=False)

    xqT = nc.declare_dram_parameter("xqT", [HID, N], F16, isOutput=False).ap()
    xkT = nc.declare_dram_parameter("xkT", [HID, N], F16, isOutput=False).ap()
    xvT = nc.declare_dram_parameter("xvT", [HID, N], F16, isOutput=False).ap()
    wqT = nc.declare_dram_parameter("wqT", [HID, HD], F16, isOutput=False).ap()
    wkT = nc.declare_dram_parameter("wkT", [HID, HD], F16, isOutput=False).ap()
    wvT = nc.declare_dram_parameter("wvT", [HID, HD], F16, isOutput=False).ap()
    woT = nc.declare_dram_parameter("woT", [HD, HID], F16, isOutput=False).ap()
    cos_d = nc.declare_dram_parameter("cos", [P, N], F16, isOutput=False).ap()
    sin_d = nc.declare_dram_parameter("sin", [P, N], F16, isOutput=False).ap()
    pm_d = nc.declare_dram_parameter("pm", [P, P], F16, isOutput=False).ap()
    msk_d = nc.declare_dram_parameter("masks", [P, nvar * 2 * QCH], F16,
                                      isOutput=False).ap()
    padb_d = nc.declare_dram_parameter("padb", [P, NKC], F32, isOutput=False).ap()
    bc1_d = nc.declare_dram_parameter("bc1", [1, P], F32R, isOutput=False).ap()
    y = nc.declare_dram_parameter("y", [N, HID], F32, isOutput=True).ap()

    with TileContext(nc) as tc:
        with (
            tc.tile_pool(name="const", bufs=1) as const,
            tc.tile_pool(name="persist", bufs=1) as persist,
            tc.tile_pool(name="xstrip", bufs=2) as xpool,
            tc.tile_pool(name="qpool", bufs=2) as qpool,
            tc.tile_pool(name="aopool", bufs=2) as aopool,
            tc.tile_pool(name="work", bufs=2) as work,
            tc.tile_pool(name="ypool", bufs=2) as ypool,
            tc.tile_pool(name="ppool", bufs=5) as ppool,
            tc.tile_pool(name="pj", bufs=2, space="PSUM") as pj,
            tc.tile_pool(name="sp", bufs=2, space="PSUM") as sppool,
            tc.tile_pool(name="av", bufs=1, space="PSUM") as avpool,
        ):
            # ---------------- weights + first strips, split per cc chunk so
            # the first projection matmuls can start as early as possible
            wv = const.tile([P, CC, HD], F16, tag="wv", name="wv")
            wk = const.tile([P, CC, HD], F16, tag="wk", name="wk")
            wq = const.tile([P, CC, HD], F16, tag="wq", name="wq")

            def load_strip(ic, halves=1):
                # halves>1 splits each tensor's DMA for earlier first-chunk
                # availability; whole-tile transfers amortize the ~400ns
                # per-descriptor DGE overhead best
                tiles = {}
                hcc = CC // halves
                for nm, dram in (("xv", xvT), ("xk", xkT), ("xq", xqT)):
                    t = xpool.tile([P, CC, ICH], F16, tag=nm, name=nm)
                    for h in range(halves):
                        csl = slice(h * hcc, (h + 1) * hcc)
                        nc.sync.dma_start(
                            out=t[:, csl, :],
                            in_=dram.rearrange("(cc p) n -> p cc n", p=P)[
                                :, csl, ic * ICH:(ic + 1) * ICH],
                        )
                    tiles[nm] = t
                return tiles

            # Startup DMA schedule: the Activation queue carries weights and
            # tables ordered by first use (wv -> pm/rope strip 0 -> wk -> wq
            # -> attention tables -> rest), while the SP queue streams the x
            # strips (xv0, xk0, xq0, then strip 1).  cos/sin load their
            # strip-0 columns first so the k-rope never waits.
            for h in range(2):
                csl = slice(h * CC // 2, (h + 1) * CC // 2)
                nc.scalar.dma_start(
                    out=wv[:, csl, :],
                    in_=wvT.rearrange("(cc p) m -> p cc m", p=P)[:, csl, :])
            x0 = load_strip(0, halves=2)
            pmt = const.tile([P, P], F16, tag="pm", name="pm")
            nc.scalar.dma_start(out=pmt, in_=pm_d)
            cost = const.tile([P, N], F16, tag="cos", name="cos")
            sint = const.tile([P, N], F16, tag="sin", name="sin")
            nc.scalar.dma_start(out=sint[:, 0:ICH], in_=sin_d[:, 0:ICH])
            nc.scalar.dma_start(out=cost[:, 0:ICH], in_=cos_d[:, 0:ICH])
            for h in range(2):
                csl = slice(h * CC // 2, (h + 1) * CC // 2)
                nc.scalar.dma_start(
                    out=wk[:, csl, :],
                    in_=wkT.rearrange("(cc p) m -> p cc m", p=P)[:, csl, :])
            nc.scalar.dma_start(
                out=wq, in_=wqT.rearrange("(cc p) m -> p cc m", p=P))
            padb = const.tile([P, NKC], F32, tag="padb", name="padb")
            nc.scalar.dma_start(out=padb, in_=padb_d)
            bc1 = const.tile([1, P], F32R, tag="bc1", name="bc1")
            nc.scalar.dma_start(out=bc1, in_=bc1_d)
            mskt = const.tile([P, nvar * 2 * QCH], F16, tag="masks", name="mskt")
            nc.scalar.dma_start(out=mskt, in_=msk_d)
            nc.scalar.dma_start(out=sint[:, ICH:], in_=sin_d[:, ICH:])
            nc.scalar.dma_start(out=cost[:, ICH:], in_=cos_d[:, ICH:])
            wo = const.tile([P, PAIRS, HID], F16, tag="wo", name="wo")
            nc.scalar.dma_start(out=wo, in_=woT.rearrange("(jc p) o -> p jc o", p=P))

            # ---------------- persistent activations
            kT = [persist.tile([P, N], F16, tag=f"kT{p}", name=f"kT{p}")
                  for p in range(PAIRS)]
            # v layout: [kpos, kc, head, d+1]; column 64 is the constant
            # ones column that accumulates the softmax denominator.
            vt = persist.tile([P, NKC, HPG, D + 1], F16, tag="vt", name="vt")
            nc.vector.memset(vt[:, :, :, D], 1.0)
            if streaming:
                qT = None
            else:
                qT = [persist.tile([P, N], F16, tag=f"qT{p}", name=f"qT{p}")
                      for p in range(PAIRS)]

            # ------------- unit builders (each unit emits a few instructions)
            def v_chain_units(x_sb, sub, ic4):
                st = {}
                units = []

                def mm(cc):
                    def f():
                        if cc == 0:
                            st["ps"] = pj.tile([P, HD], F32, tag="mm",
                                               name="pvmm")
                        nc.tensor.matmul(
                            st["ps"],
                            lhsT=x_sb[:, cc, sub * P:(sub + 1) * P],
                            rhs=wv[:, cc, :],
                            start=(cc == 0), stop=(cc == CC - 1),
                        )
                    return f

                def fin():
                    kc = ic4 * (ICH // P) + sub
                    nc.vector.tensor_copy(
                        out=vt[:, kc, :, 0:D],
                        in_=st["ps"][:].rearrange("p (h d) -> p h d", h=HPG),
                    )
                return [mm(cc) for cc in range(CC)], [fin]

            def qk_chain_units(x_sb, w_sb, mc, ic, dst_fn):
                st = {}
                icsl = slice(ic * ICH, (ic + 1) * ICH)

                def mm(cc):
                    def f():
                        if cc == 0:
                            st["ps"] = pj.tile([P, ICH], F32, tag="mm",
                                               name="pjmm")
                        nc.tensor.matmul(
                            st["ps"],
                            lhsT=w_sb[:, cc, mc * P:(mc + 1) * P],
                            rhs=x_sb[:, cc, :],
                            start=(cc == 0), stop=(cc == CC - 1),
                        )
                    return f

                def raw_copy():
                    st["raw"] = work.tile([P, ICH], F16, tag="raw", name="raw")
                    nc.vector.tensor_copy(out=st["raw"], in_=st["ps"])

                def sin_mul():
                    # sin/cos are equal within each rotation pair, so
                    # rot(x) * sin == rot(x * sin): multiply before the
                    # rotation and the rope add can read the rotation's
                    # PSUM output directly.
                    st["rawsin"] = work.tile([P, ICH], F16, tag="ropetmp",
                                             name="rawsin")
                    nc.vector.tensor_mul(st["rawsin"], st["raw"],
                                         sint[:, icsl])

                def rot_mm():
                    # the rotation overwrites the projection's own PSUM
                    # tile (free after raw_copy/sin_mul) - no extra bank
                    nc.tensor.matmul(st["ps"], lhsT=pmt, rhs=st["rawsin"],
                                     start=True, stop=True)

                def rope_muls():
                    dsth = dst_fn()
                    nc.vector.tensor_mul(dsth, st["raw"], cost[:, icsl])
                    nc.vector.tensor_add(dsth, dsth, st["ps"])

                return [mm(cc) for cc in range(CC)], [raw_copy, sin_mul,
                                                      rot_mm, rope_muls]

            def merge_chains(chains):
                """Serialize chains; each chain's finish units are slotted
                into the next chain's matmul stream with enough PE time
                between dependent units."""
                units = []
                pending = []
                for mms, fins in chains:
                    for j, m in enumerate(mms):
                        if j == 0 and pending:
                            units.append(pending.pop(0))
                        units.append(m)
                        if j in (1, 5, 6) and pending:
                            units.append(pending.pop(0))
                    pending = list(fins)
                units += pending
                return units

            def proj_strip_units(x_tiles, ic, q_dst, q_persistent=False):
                """All projection units for one strip: v,k,q waves, with
                chains pair-interleaved so dependent units get spacing."""
                vs = [v_chain_units(x_tiles["xv"], sub, ic)
                      for sub in range(ICH // P)]
                ks = [qk_chain_units(
                        x_tiles["xk"], wk, mc, ic,
                        (lambda mc=mc: kT[mc][:, ic * ICH:(ic + 1) * ICH]))
                      for mc in range(PAIRS)]

                def q_dst_fn(mc):
                    def f():
                        if q_persistent:
                            q_dst[mc] = qT[mc][:, ic * ICH:(ic + 1) * ICH]
                        else:
                            q_dst[mc] = qpool.tile([P, ICH], F16,
                                                   tag=f"qs{mc}",
                                                   name=f"qs{mc}")
                        return q_dst[mc]
                    return f
                qs = [qk_chain_units(x_tiles["xq"], wq, mc, ic, q_dst_fn(mc))
                      for mc in range(PAIRS)]
                # head: everything attention pairs 0/1 need; tail: pairs
                # 2/3's k+q chains, deferred into the next attention window
                # to fill its exp-bound stretches with PE work
                flags = {0: True, 1: True}

                def mark(pp):
                    def f():
                        flags[pp] = True
                    return f
                head = merge_chains(vs + [ks[0], qs[0], ks[1], qs[1]])
                tail = (merge_chains([ks[2], qs[2]]) + [mark(2)]
                        + merge_chains([ks[3], qs[3]]) + [mark(3)])
                return head, tail, flags

            def outproj_units(ic, ao_tiles, spread=False):
                chains = []
                for sub in range(ICH // P):
                    i128 = ic * (ICH // P) + sub
                    isl = slice(sub * P, (sub + 1) * P)
                    for oc in range(HID // 512):
                        osl = slice(oc * 512, (oc + 1) * 512)
                        st = {}
                        # at the drain (last strip) the attention PSUM pools
                        # are free; alternating into them avoids pj WAR stalls
                        pool, tg = ((sppool, "sp2") if spread and oc else
                                    (pj, "mm"))

                        def mm(pp, st=st, isl=isl, osl=osl, pool=pool, tg=tg):
                            def f():
                                if pp == 0:
                                    st["ps"] = pool.tile([P, 512], F32,
                                                         tag=tg, name="yps")
                                nc.tensor.matmul(
                                    st["ps"],
                                    lhsT=ao_tiles[pp][:, isl],
                                    rhs=wo[:, pp, osl],
                                    start=(pp == 0), stop=(pp == PAIRS - 1),
                                )
                            return f

                        def fin(st=st, i128=i128, osl=osl):
                            yt = ypool.tile([P, 512], F32, tag="yout",
                                            name="yt")
                            nc.vector.tensor_copy(out=yt, in_=st["ps"])
                            nc.sync.dma_start(
                                out=y[i128 * P:(i128 + 1) * P, osl], in_=yt)
                        chains.append(([mm(pp) for pp in range(PAIRS)],
                                       [fin]))
                return merge_chains(chains)

            # ------------- the filler queue
            queue = []
            qpos = [0]

            def pump(n):
                end = min(qpos[0] + n, len(queue))
                while qpos[0] < end:
                    queue[qpos[0]]()
                    qpos[0] += 1

            def pump_all():
                pump(len(queue) - qpos[0])

            def attn_block(pp, qc, q_tile, ao_tile, pacer):
                """Attention for head pair pp over q chunk qc; calls
                pacer() once per k-block to pump filler units."""
                kcs = [kc for kc in range(NKC) if plan[(qc, kc)][0] != "skip"]
                if not kcs:
                    return
                av2 = avpool.tile([P, 2 * QCH], F32, tag="av2", name="av2")
                pts = {}

                def emit_av(i):
                    first, last = i == 0, i == len(kcs) - 1
                    kc = kcs[i]
                    pt2 = pts.pop(i)
                    for hh in range(2):
                        nc.tensor.matmul(
                            av2[0:D + 1, hh * QCH:(hh + 1) * QCH],
                            lhsT=vt[:, kc, 2 * pp + hh, :],
                            rhs=pt2[:, hh * QCH:(hh + 1) * QCH],
                            start=first, stop=last,
                        )

                for idx, kc in enumerate(kcs):
                    ksl = slice(kc * KCH, (kc + 1) * KCH)
                    kind, var = plan[(qc, kc)]
                    sp2 = sppool.tile([P, 2 * QCH], F32, tag="sp2", name="sp2")
                    for hh in range(2):
                        base = hh * D
                        nc.tensor.matmul(
                            sp2[:, hh * QCH:(hh + 1) * QCH],
                            lhsT=kT[pp][base:base + D, ksl],
                            rhs=q_tile[base:base + D, :],
                            start=True, stop=True,
                            tile_position=(base, 0),
                        )
                    pt2 = ppool.tile([P, 2 * QCH], F16, tag="pt", name="pt")
                    if TWOBANK:
                        nc.scalar.activation(
                            pt2, sp2, mybir.ActivationFunctionType.Exp,
                            scale=SCALE, bias=padb[:, kc:kc + 1])
                    else:
                        for hh in range(2):
                            hsl = slice(hh * QCH, (hh + 1) * QCH)
                            nc.scalar.activation(
                                pt2[:, hsl], sp2[:, hsl],
                                mybir.ActivationFunctionType.Exp,
                                scale=SCALE, bias=padb[:, kc:kc + 1])
                    if kind == "partial":
                        nc.vector.tensor_mul(
                            pt2, pt2,
                            mskt[:, var * 2 * QCH:(var + 1) * 2 * QCH])
                    pts[idx] = pt2
                    pacer()
                    if idx >= LAG:
                        emit_av(idx - LAG)
                for i in range(max(0, len(kcs) - LAG), len(kcs) - 1):
                    emit_av(i)
                pump(3)
                emit_av(len(kcs) - 1)

                # normalize.  reciprocal of the denominator row + copies of
                # the two av halves come off PSUM first (one PSUM operand
                # per op - NCC_IBVF027 - and av2 frees for the next pair as
                # early as possible); then a K=1 ones matmul broadcasts the
                # reciprocal and DVE multiplies into ao.
                dnr = work.tile([1, 2 * QCH], F32R, tag="dnr", name="dnr")
                with nc.allow_low_precision(reason="f32r == f32 bitwise"):
                    if TWOBANK:
                        nc.vector.reciprocal(dnr, av2[D:D + 1, :])
                    else:
                        for hh in range(2):
                            hsl = slice(hh * QCH, (hh + 1) * QCH)
                            nc.vector.reciprocal(dnr[:, hsl],
                                                 av2[D:D + 1, hsl])
                av_sb = work.tile([D, 2 * QCH], F32, tag="avsb",
                                  name="av_sb")
                nc.vector.tensor_copy(out=av_sb, in_=av2[0:D, :])
                pump(4)
                for hh in range(2):
                    bcp = pj.tile([P, QCH], F32, tag="mm", name="bcps")
                    nc.tensor.matmul(bcp[0:D, :], lhsT=bc1[:, 0:D],
                                     rhs=dnr[:, hh * QCH:(hh + 1) * QCH],
                                     start=True, stop=True)
                    nc.vector.tensor_mul(
                        ao_tile[hh * D:(hh + 1) * D, :],
                        av_sb[:, hh * QCH:(hh + 1) * QCH],
                        bcp[0:D, :])

            for _rep in range(REPEAT):
                if streaming:
                    q_cur, q_nxt = {}, {}
                    # strip 1's x loads go out before strip 0's compute so
                    # its projection chains never wait on DMA
                    x_nxt = load_strip(1)
                    h0, t0, fl_cur = proj_strip_units(x0, 0, q_cur)
                    queue += h0 + t0
                    pump_all()
                    tail_cur = None
                    for ic in range(NSTRIP):
                        if ic + 2 < NSTRIP:
                            x_fut = load_strip(ic + 2)
                        else:
                            x_fut = None
                        if tail_cur:
                            # deferred pair-2/3 chains of this strip run
                            # first in this attention window
                            queue[qpos[0]:qpos[0]] = tail_cur
                        if ic + 1 < NSTRIP:
                            h2, t2, fl2 = proj_strip_units(x_nxt, ic + 1,
                                                           q_nxt)
                            queue += h2
                        else:
                            t2, fl2 = None, None
                        ao_tiles = [aopool.tile([P, QCH], F16, tag=f"aos{pp}",
                                                name=f"aos{pp}")
                                    for pp in range(PAIRS)]
                        # spread the queued units evenly over this strip's
                        # k-blocks (fractional pacing)
                        nslots = sum(
                            1 for pp in range(PAIRS) for kc in range(NKC)
                            if plan[(ic, kc)][0] != "skip")
                        todo = len(queue) - qpos[0]
                        state = {"slot": 0, "base": qpos[0]}

                        def pacer(todo=todo, nslots=max(nslots, 1),
                                  state=state):
                            state["slot"] += 1
                            target = state["base"] + (
                                state["slot"] * todo + nslots - 1) // nslots
                            pump(max(0, target - qpos[0]))
                        for pp in range(PAIRS):
                            guard = 0
                            while not fl_cur.get(pp):
                                pump(2)
                                guard += 1
                                assert guard < 10000, "tail never emitted"
                            attn_block(pp, ic, q_cur[pp], ao_tiles[pp],
                                       pacer)
                        pump_all()
                        queue += outproj_units(ic, ao_tiles,
                                               spread=(ic == NSTRIP - 1))
                        q_cur, q_nxt = q_nxt, {}
                        x_nxt = x_fut
                        tail_cur, fl_cur = t2, fl2
                    pump_all()
                    if REPEAT > 1 and _rep < REPEAT - 1:
                        x0 = load_strip(0)
                else:
                    q_all = [{} for _ in range(NSTRIP)]
                    for ic in range(NSTRIP):
                        xs = x0 if ic == 0 else load_strip(ic)
                        hh_, tt_, _fl = proj_strip_units(xs, ic, q_all[ic],
                                                         q_persistent=True)
                        queue += hh_ + tt_
                        pump_all()
                    for qc in range(NQC):
                        ao_tiles = [aopool.tile([P, QCH], F16, tag=f"aos{pp}",
                                                name=f"aos{pp}")
                                    for pp in range(PAIRS)]
                        for pp in range(PAIRS):
                            attn_block(pp, qc, qT[pp][:, qc * QCH:(qc + 1) * QCH],
                                       ao_tiles[pp], lambda: None)
                        queue += outproj_units(qc, ao_tiles)
                        pump_all()
                    if REPEAT > 1 and _rep < REPEAT - 1:
                        x0 = load_strip(0)

    nc.compile()
    return nc, masks_np


def _get_nc(is_causal, start_pos):
    key = (bool(is_causal), int(start_pos), REPEAT)
    if key not in _NC_CACHE:
        _NC_CACHE[key] = _build_nc(bool(is_causal), int(start_pos))
    return _NC_CACHE[key]


# ---------------------------------------------------------------- entry
def kernel(x_q, x_k, x_v, W_q, W_k, W_v, W_out, padding_mask, is_causal,
           start_pos):
    x_q = np.asarray(x_q, dtype=np.float32)
    x_k = np.asarray(x_k, dtype=np.float32)
    x_v = np.asarray(x_v, dtype=np.float32)
    W_q = np.asarray(W_q, dtype=np.float32)
    W_k = np.asarray(W_k, dtype=np.float32)
    W_v = np.asarray(W_v, dtype=np.float32)
    W_out = np.asarray(W_out, dtype=np.float32)
    padding_mask = np.asarray(padding_mask).astype(bool)
    is_causal = int(np.asarray(is_causal))
    start_pos = int(np.asarray(start_pos))

    nc, masks = _get_nc(is_causal, start_pos)

    cos2, sin2 = _rope_tables()
    pm = _perm_matrix()

    in_maps = []
    for c in range(NCORES):
        bi, hg = divmod(c, GROUPS)
        hs = hg * HD
        pad = padding_mask[bi]
        padb = np.where(pad.reshape(NKC, P).T, ESHIFT, ESHIFT - 30.0)
        in_maps.append({
            "xqT": np.ascontiguousarray(x_q[bi].T).astype(np.float16),
            "xkT": np.ascontiguousarray(x_k[bi].T).astype(np.float16),
            "xvT": np.ascontiguousarray(x_v[bi].T).astype(np.float16),
            "wqT": np.ascontiguousarray(W_q[hs:hs + HD].T).astype(np.float16),
            "wkT": np.ascontiguousarray(W_k[hs:hs + HD].T).astype(np.float16),
            "wvT": np.ascontiguousarray(W_v[hs:hs + HD].T).astype(np.float16),
            "woT": np.ascontiguousarray(W_out[:, hs:hs + HD].T).astype(np.float16),
            "cos": cos2,
            "sin": sin2,
            "pm": pm,
            "masks": masks,
            "padb": np.ascontiguousarray(padb, dtype=np.float32),
            "bc1": np.ones((1, P), dtype=np.float32),
        })

    res = run_bass_kernel_spmd(nc, in_maps, list(range(NCORES)))
    out = np.empty((B, N, HID), dtype=np.float32)
    for bi in range(B):
        out[bi] = res.results[GROUPS * bi]["y"]
        for g in range(1, GROUPS):
            out[bi] += res.results[GROUPS * bi + g]["y"]
    return out


# revision 63
# speedup vs baseline: 2.1966x; 2.1966x over previous
"""Multi-head attention (RoPE, causal) Trainium2 Bass kernel.

Sharding (8 cores): data-parallel over batch (4) x tensor-parallel over
heads (16 -> 2 groups of 8).  Core c handles batch c//2 and head group
c%2.  Attention is fully head-local; the out-projection partial sums of
the two head groups of each batch are added on the host.

v3 design (sizes hardcoded for b=4, n=2048, hidden=1024, h=16, d=64):
  - fp16 end to end (PSUM accumulation stays f32).  The softmax scale
    1/8 and a constant -3 shift (so p fits fp16 comfortably) fold into
    the exp activation; the shift cancels in the normalization.
  - Padding folds into the exp bias (per-k-partition scalar), so the
    denominator ones-column in v is constant and v needs no scaling.
  - Both heads of a pair write score PSUM banks side by side in one
    [128,1024] 2-bank tile -> a single exp instruction per (pair,kc).
  - Causal masking multiplies the exp output (diagonal blocks only).
  - AV lags the exp by two k-blocks so the PE never waits on ScalarE.
  - Software pipelining: the projection matmuls of strip ic+1 and the
    out-projection of strip ic-1 are chopped into single-matmul units
    and pumped into the attention instruction stream, so the PE stays
    busy through the exp-bound attention inner loop.
  - RoPE: sin/cos are equal within each rotation pair, so the sin
    multiply happens before the pair-permutation matmul (whose output
    accumulates in the projection's own PSUM tile); all elementwise
    work is fp16 on DVE.
  - Startup DMAs are ordered by first use across two hardware queues
    (SP: x strips, Activation: weights/tables), sized to amortize the
    per-descriptor DGE overhead.
"""

import numpy as np

import concourse.bass as bass
import concourse.mybir as mybir
from concourse import bacc
from concourse.tile import TileContext
from concourse.bass_utils import run_bass_kernel_spmd

# ---------------------------------------------------------------- constants
B, N, HID = 4, 2048, 1024
H = 16
D = HID // H                     # 64
NCORES = 8
GROUPS = NCORES // B             # 2 head groups
HPG = H // GROUPS                # 8 heads per core
HD = HPG * D                     # 512 local head dims
PAIRS = HPG // 2                 # 4 head pairs per core
ROPE_THETA = 10000.0
SCALE = 0.125                    # 1/sqrt(d)
ESHIFT = -3.0                    # exp bias shift, cancels in normalization

P = 128
CC = HID // P                    # 8 contraction chunks for projections
ICH = 512                        # projection i-chunk (moving free dim)
QCH = 512                        # attention q-chunk
KCH = 128                        # attention k-chunk
NQC = N // QCH                   # 4
NKC = N // KCH                   # 16
NSTRIP = N // ICH                # 4

F32 = mybir.dt.float32
F32R = mybir.dt.float32r
F16 = mybir.dt.float16

_NC_CACHE = {}
REPEAT = 1
LAG = 4
TWOBANK = True    # single [128,1024] exp spanning 2 PSUM banks per (pair,kc)


# ---------------------------------------------------------------- host prep
def _allow_matrix(is_causal, start_pos):
    i = np.arange(N)[:, None]    # query index
    j = np.arange(N)[None, :]    # key index
    if is_causal:
        return (j < start_pos) | ((i >= start_pos) & (i >= j))
    return np.ones((N, N), dtype=bool)


def _block_plan(is_causal, start_pos):
    """Classify each (qc, kc) score block; return plan + mask table.

    plan[(qc, kc)] is 'skip', 'full', or ('partial', variant_idx).
    The mask table holds, per variant, a [128, 2*QCH] 0/1 fp16 block
    (the same [128, QCH] mask duplicated for the two heads of a pair).
    """
    allow = _allow_matrix(is_causal, start_pos)
    plan = {}
    variants = []
    vkeys = {}
    for qc in range(NQC):
        for kc in range(NKC):
            blk = allow[qc * QCH:(qc + 1) * QCH, kc * KCH:(kc + 1) * KCH]
            if not blk.any():
                plan[(qc, kc)] = ("skip", None)
                continue
            if blk.all():
                plan[(qc, kc)] = ("full", None)
                continue
            bT = blk.T               # [128 k, 512 q]
            start = np.argmax(bT, axis=1)
            for r in range(KCH):
                if not bT[r].any():
                    raise NotImplementedError("empty k-row in partial block")
                s = start[r]
                if not bT[r, s:].all() or bT[r, :s].any():
                    raise NotImplementedError("non-suffix mask row")
            key = start.tobytes()
            if key not in vkeys:
                vkeys[key] = len(variants)
                variants.append(start.astype(np.float32))
            plan[(qc, kc)] = ("partial", vkeys[key])
    if not variants:
        variants.append(np.zeros(KCH, dtype=np.float32))
    q = np.arange(QCH)[None, :]
    blocks = []
    for v in variants:
        m = (q >= v[:, None]).astype(np.float16)      # [128, QCH]
        blocks.append(np.concatenate([m, m], axis=1))  # [128, 2*QCH]
    masks = np.concatenate(blocks, axis=1)             # [128, V*2*QCH]
    return plan, masks


def _rope_tables():
    inv_freq = 1.0 / (ROPE_THETA ** (np.arange(0, D, 2, dtype=np.float64) / D))
    t = np.arange(N, dtype=np.float64)
    freqs = t[:, None] * inv_freq[None, :]        # [N, 32]
    freqs = np.repeat(freqs, 2, axis=1)           # [N, 64]
    cos = np.cos(freqs).T.astype(np.float16)      # [64, N]
    sin = np.sin(freqs).T.astype(np.float16)
    cos2 = np.concatenate([cos, cos], axis=0)     # [128, N]
    sin2 = np.concatenate([sin, sin], axis=0)
    return np.ascontiguousarray(cos2), np.ascontiguousarray(sin2)


def _perm_matrix():
    # rot = PM @ q  with rot[2r] = -q[2r+1], rot[2r+1] = q[2r].
    # matmul computes lhsT.T @ rhs, so pass PM.T.
    pm = np.zeros((P, P), dtype=np.float16)
    for r in range(P // 2):
        pm[2 * r, 2 * r + 1] = -1.0
        pm[2 * r + 1, 2 * r] = 1.0
    return np.ascontiguousarray(pm.T)


# ---------------------------------------------------------------- device IR
def _build_nc(is_causal, start_pos):
    plan, masks_np = _block_plan(is_causal, start_pos)
    nvar = masks_np.shape[1] // (2 * QCH)
    streaming = bool(is_causal)

    nc = bacc.Bacc("TRN2", target_bir_lowering=False, debug=False)

    xqT = nc.declare_dram_parameter("xqT", [HID, N], F16, isOutput=False).ap()
    xkT = nc.declare_dram_parameter("xkT", [HID, N], F16, isOutput=False).ap()
    xvT = nc.declare_dram_parameter("xvT", [HID, N], F16, isOutput=False).ap()
    wqT = nc.declare_dram_parameter("wqT", [HID, HD], F16, isOutput=False).ap()
    wkT = nc.declare_dram_parameter("wkT", [HID, HD], F16, isOutput=False).ap()
    wvT = nc.declare_dram_parameter("wvT", [HID, HD], F16, isOutput=False).ap()
    woT = nc.declare_dram_parameter("woT", [HD, HID], F16, isOutput=False).ap()
    cos_d = nc.declare_dram_parameter("cos", [P, N], F16, isOutput=False).ap()
    sin_d = nc.declare_dram_parameter("sin", [P, N], F16, isOutput=False).ap()
    pm_d = nc.declare_dram_parameter("pm", [P, P], F16, isOutput=False).ap()
    msk_d = nc.declare_dram_parameter("masks", [P, nvar * 2 * QCH], F16,
                                      isOutput=False).ap()
    padb_d = nc.declare_dram_parameter("padb", [P, NKC], F32, isOutput=False).ap()
    bc1_d = nc.declare_dram_parameter("bc1", [1, P], F32R, isOutput=False).ap()
    y = nc.declare_dram_parameter("y", [N, HID], F32, isOutput=True).ap()

    with TileContext(nc) as tc:
        with (
            tc.tile_pool(name="const", bufs=1) as const,
            tc.tile_pool(name="persist", bufs=1) as persist,
            tc.tile_pool(name="xstrip", bufs=2) as xpool,
            tc.tile_pool(name="qpool", bufs=2) as qpool,
            tc.tile_pool(name="aopool", bufs=2) as aopool,
            tc.tile_pool(name="work", bufs=2) as work,
            tc.tile_pool(name="ypool", bufs=2) as ypool,
            tc.tile_pool(name="ppool", bufs=5) as ppool,
            tc.tile_pool(name="pj", bufs=2, space="PSUM") as pj,
            tc.tile_pool(name="sp", bufs=2, space="PSUM") as sppool,
            tc.tile_pool(name="av", bufs=1, space="PSUM") as avpool,
        ):
            # ---------------- weights + first strips, split per cc chunk so
            # the first projection matmuls can start as early as possible
            wv = const.tile([P, CC, HD], F16, tag="wv", name="wv")
            wk = const.tile([P, CC, HD], F16, tag="wk", name="wk")
            wq = const.tile([P, CC, HD], F16, tag="wq", name="wq")

            def load_strip(ic, halves=1):
                # halves>1 splits each tensor's DMA for earlier first-chunk
                # availability; whole-tile transfers amortize the ~400ns
                # per-descriptor DGE overhead best
                tiles = {}
                hcc = CC // halves
                for nm, dram in (("xv", xvT), ("xk", xkT), ("xq", xqT)):
                    t = xpool.tile([P, CC, ICH], F16, tag=nm, name=nm)
                    for h in range(halves):
                        csl = slice(h * hcc, (h + 1) * hcc)
                        nc.sync.dma_start(
                            out=t[:, csl, :],
                            in_=dram.rearrange("(cc p) n -> p cc n", p=P)[
                                :, csl, ic * ICH:(ic + 1) * ICH],
                        )
                    tiles[nm] = t
                return tiles

            # Startup DMA schedule: the Activation queue carries weights and
            # tables ordered by first use (wv -> pm/rope strip 0 -> wk -> wq
            # -> attention tables -> rest), while the SP queue streams the x
            # strips (xv0, xk0, xq0, then strip 1).  cos/sin load their
            # strip-0 columns first so the k-rope never waits.
            for csl in (slice(0, 2), slice(2, 5), slice(5, CC)):
                nc.scalar.dma_start(
                    out=wv[:, csl, :],
                    in_=wvT.rearrange("(cc p) m -> p cc m", p=P)[:, csl, :])
            x0 = load_strip(0, halves=4)
            pmt = const.tile([P, P], F16, tag="pm", name="pm")
            nc.scalar.dma_start(out=pmt, in_=pm_d)
            cost = const.tile([P, N], F16, tag="cos", name="cos")
            sint = const.tile([P, N], F16, tag="sin", name="sin")
            nc.scalar.dma_start(out=sint[:, 0:ICH], in_=sin_d[:, 0:ICH])
            nc.scalar.dma_start(out=cost[:, 0:ICH], in_=cos_d[:, 0:ICH])
            for h in range(2):
                csl = slice(h * CC // 2, (h + 1) * CC // 2)
                nc.scalar.dma_start(
                    out=wk[:, csl, :],
                    in_=wkT.rearrange("(cc p) m -> p cc m", p=P)[:, csl, :])
            nc.scalar.dma_start(
                out=wq, in_=wqT.rearrange("(cc p) m -> p cc m", p=P))
            padb = const.tile([P, NKC], F32, tag="padb", name="padb")
            nc.scalar.dma_start(out=padb, in_=padb_d)
            bc1 = const.tile([1, P], F32R, tag="bc1", name="bc1")
            nc.scalar.dma_start(out=bc1, in_=bc1_d)
            mskt = const.tile([P, nvar * 2 * QCH], F16, tag="masks", name="mskt")
            nc.scalar.dma_start(out=mskt, in_=msk_d)
            nc.scalar.dma_start(out=sint[:, ICH:], in_=sin_d[:, ICH:])
            nc.scalar.dma_start(out=cost[:, ICH:], in_=cos_d[:, ICH:])
            wo = const.tile([P, PAIRS, HID], F16, tag="wo", name="wo")
            nc.scalar.dma_start(out=wo, in_=woT.rearrange("(jc p) o -> p jc o", p=P))

            # ---------------- persistent activations
            kT = [persist.tile([P, N], F16, tag=f"kT{p}", name=f"kT{p}")
                  for p in range(PAIRS)]
            # v layout: [kpos, kc, head, d+1]; column 64 is the constant
            # ones column that accumulates the softmax denominator.
            vt = persist.tile([P, NKC, HPG, D + 1], F16, tag="vt", name="vt")
            nc.vector.memset(vt[:, :, :, D], 1.0)
            if streaming:
                qT = None
            else:
                qT = [persist.tile([P, N], F16, tag=f"qT{p}", name=f"qT{p}")
                      for p in range(PAIRS)]

            # ------------- unit builders (each unit emits a few instructions)
            def v_chain_units(x_sb, sub, ic4):
                st = {}
                units = []

                def mm(cc):
                    def f():
                        if cc == 0:
                            st["ps"] = pj.tile([P, HD], F32, tag="mm",
                                               name="pvmm")
                        nc.tensor.matmul(
                            st["ps"],
                            lhsT=x_sb[:, cc, sub * P:(sub + 1) * P],
                            rhs=wv[:, cc, :],
                            start=(cc == 0), stop=(cc == CC - 1),
                        )
                    return f

                def fin():
                    kc = ic4 * (ICH // P) + sub
                    nc.vector.tensor_copy(
                        out=vt[:, kc, :, 0:D],
                        in_=st["ps"][:].rearrange("p (h d) -> p h d", h=HPG),
                    )
                return [mm(cc) for cc in range(CC)], [fin]

            def qk_chain_units(x_sb, w_sb, mc, ic, dst_fn):
                st = {}
                icsl = slice(ic * ICH, (ic + 1) * ICH)

                def mm(cc):
                    def f():
                        if cc == 0:
                            st["ps"] = pj.tile([P, ICH], F32, tag="mm",
                                               name="pjmm")
                        nc.tensor.matmul(
                            st["ps"],
                            lhsT=w_sb[:, cc, mc * P:(mc + 1) * P],
                            rhs=x_sb[:, cc, :],
                            start=(cc == 0), stop=(cc == CC - 1),
                        )
                    return f

                def raw_copy():
                    st["raw"] = work.tile([P, ICH], F16, tag="raw", name="raw")
                    nc.vector.tensor_copy(out=st["raw"], in_=st["ps"])

                def sin_mul():
                    # sin/cos are equal within each rotation pair, so
                    # rot(x) * sin == rot(x * sin): multiply before the
                    # rotation and the rope add can read the rotation's
                    # PSUM output directly.
                    st["rawsin"] = work.tile([P, ICH], F16, tag="ropetmp",
                                             name="rawsin")
                    nc.vector.tensor_mul(st["rawsin"], st["raw"],
                                         sint[:, icsl])

                def rot_mm():
                    # the rotation overwrites the projection's own PSUM
                    # tile (free after raw_copy/sin_mul) - no extra bank
                    nc.tensor.matmul(st["ps"], lhsT=pmt, rhs=st["rawsin"],
                                     start=True, stop=True)

                def rope_muls():
                    dsth = dst_fn()
                    nc.vector.tensor_mul(dsth, st["raw"], cost[:, icsl])
                    nc.vector.tensor_add(dsth, dsth, st["ps"])

                return [mm(cc) for cc in range(CC)], [raw_copy, sin_mul,
                                                      rot_mm, rope_muls]

            def merge_chains(chains):
                """Serialize chains; each chain's finish units are slotted
                into the next chain's matmul stream with enough PE time
                between dependent units."""
                units = []
                pending = []
                for mms, fins in chains:
                    for j, m in enumerate(mms):
                        if j == 0 and pending:
                            units.append(pending.pop(0))
                        units.append(m)
                        if j in (1, 5, 6) and pending:
                            units.append(pending.pop(0))
                    pending = list(fins)
                units += pending
                return units

            def proj_strip_units(x_tiles, ic, q_dst, q_persistent=False):
                """All projection units for one strip: v,k,q waves, with
                chains pair-interleaved so dependent units get spacing."""
                vs = [v_chain_units(x_tiles["xv"], sub, ic)
                      for sub in range(ICH // P)]
                ks = [qk_chain_units(
                        x_tiles["xk"], wk, mc, ic,
                        (lambda mc=mc: kT[mc][:, ic * ICH:(ic + 1) * ICH]))
                      for mc in range(PAIRS)]

                def q_dst_fn(mc):
                    def f():
                        if q_persistent:
                            q_dst[mc] = qT[mc][:, ic * ICH:(ic + 1) * ICH]
                        else:
                            q_dst[mc] = qpool.tile([P, ICH], F16,
                                                   tag=f"qs{mc}",
                                                   name=f"qs{mc}")
                        return q_dst[mc]
                    return f
                qs = [qk_chain_units(x_tiles["xq"], wq, mc, ic, q_dst_fn(mc))
                      for mc in range(PAIRS)]
                # head: everything attention pairs 0/1 need; tail: pairs
                # 2/3's k+q chains, deferred into the next attention window
                # to fill its exp-bound stretches with PE work
                flags = {0: True, 1: True}

                def mark(pp):
                    def f():
                        flags[pp] = True
                    return f
                head = merge_chains(vs + [ks[0], qs[0], ks[1], qs[1]])
                tail = (merge_chains([ks[2], qs[2]]) + [mark(2)]
                        + merge_chains([ks[3], qs[3]]) + [mark(3)])
                return head, tail, flags

            def outproj_units(ic, ao_tiles, spread=False):
                chains = []
                for sub in range(ICH // P):
                    i128 = ic * (ICH // P) + sub
                    isl = slice(sub * P, (sub + 1) * P)
                    for oc in range(HID // 512):
                        osl = slice(oc * 512, (oc + 1) * 512)
                        st = {}
                        # at the drain (last strip) the attention PSUM pools
                        # are free and ScalarE is idle: cycle the yps tiles
                        # over all pools and route y copies to ScalarE, away
                        # from the pair-3 norm traffic on pj/DVE
                        ci = len(chains)
                        pool, tg = ([(sppool, "sp2"), (pj, "mm"),
                                     (avpool, "av2")][ci % 3] if spread else
                                    (pj, "mm"))

                        def mm(pp, st=st, isl=isl, osl=osl, pool=pool, tg=tg):
                            def f():
                                if pp == 0:
                                    st["ps"] = pool.tile([P, 512], F32,
                                                         tag=tg, name="yps")
                                nc.tensor.matmul(
                                    st["ps"],
                                    lhsT=ao_tiles[pp][:, isl],
                                    rhs=wo[:, pp, osl],
                                    start=(pp == 0), stop=(pp == PAIRS - 1),
                                )
                            return f

                        def fin(st=st, i128=i128, osl=osl):
                            yt = ypool.tile([P, 512], F32, tag="yout",
                                            name="yt")
                            if spread:
                                nc.scalar.copy(out=yt, in_=st["ps"])
                            else:
                                nc.vector.tensor_copy(out=yt, in_=st["ps"])
                            nc.sync.dma_start(
                                out=y[i128 * P:(i128 + 1) * P, osl], in_=yt)
                        # fin is Act/DVE + DMA (never blocks the PE
                        # stream): run it right after its own matmuls so
                        # the drain tail stays short
                        chains.append(([mm(pp) for pp in range(PAIRS)]
                                       + [fin], []))
                return merge_chains(chains)

            # ------------- the filler queue
            queue = []
            qpos = [0]

            def pump(n):
                end = min(qpos[0] + n, len(queue))
                while qpos[0] < end:
                    queue[qpos[0]]()
                    qpos[0] += 1

            def pump_all():
                pump(len(queue) - qpos[0])

            def attn_block(pp, qc, q_tile, ao_tile, pacer):
                """Attention for head pair pp over q chunk qc; calls
                pacer() once per k-block to pump filler units."""
                kcs = [kc for kc in range(NKC) if plan[(qc, kc)][0] != "skip"]
                if not kcs:
                    return
                av2 = avpool.tile([P, 2 * QCH], F32, tag="av2", name="av2")
                pts = {}

                def emit_av(i):
                    first, last = i == 0, i == len(kcs) - 1
                    kc = kcs[i]
                    pt2 = pts.pop(i)
                    for hh in range(2):
                        nc.tensor.matmul(
                            av2[0:D + 1, hh * QCH:(hh + 1) * QCH],
                            lhsT=vt[:, kc, 2 * pp + hh, :],
                            rhs=pt2[:, hh * QCH:(hh + 1) * QCH],
                            start=first, stop=last,
                        )

                for idx, kc in enumerate(kcs):
                    ksl = slice(kc * KCH, (kc + 1) * KCH)
                    kind, var = plan[(qc, kc)]
                    sp2 = sppool.tile([P, 2 * QCH], F32, tag="sp2", name="sp2")
                    for hh in range(2):
                        base = hh * D
                        nc.tensor.matmul(
                            sp2[:, hh * QCH:(hh + 1) * QCH],
                            lhsT=kT[pp][base:base + D, ksl],
                            rhs=q_tile[base:base + D, :],
                            start=True, stop=True,
                            tile_position=(base, 0),
                        )
                    pt2 = ppool.tile([P, 2 * QCH], F16, tag="pt", name="pt")
                    if TWOBANK:
                        nc.scalar.activation(
                            pt2, sp2, mybir.ActivationFunctionType.Exp,
                            scale=SCALE, bias=padb[:, kc:kc + 1])
                    else:
                        for hh in range(2):
                            hsl = slice(hh * QCH, (hh + 1) * QCH)
                            nc.scalar.activation(
                                pt2[:, hsl], sp2[:, hsl],
                                mybir.ActivationFunctionType.Exp,
                                scale=SCALE, bias=padb[:, kc:kc + 1])
                    if kind == "partial":
                        nc.vector.tensor_mul(
                            pt2, pt2,
                            mskt[:, var * 2 * QCH:(var + 1) * 2 * QCH])
                    pts[idx] = pt2
                    pacer()
                    if idx >= LAG:
                        emit_av(idx - LAG)
                for i in range(max(0, len(kcs) - LAG), len(kcs) - 1):
                    emit_av(i)
                pump(3)
                emit_av(len(kcs) - 1)

                # normalize.  reciprocal of the denominator row + copies of
                # the two av halves come off PSUM first (one PSUM operand
                # per op - NCC_IBVF027 - and av2 frees for the next pair as
                # early as possible); then a K=1 ones matmul broadcasts the
                # reciprocal and DVE multiplies into ao.
                dnr = work.tile([1, 2 * QCH], F32R, tag="dnr", name="dnr")
                with nc.allow_low_precision(reason="f32r == f32 bitwise"):
                    if TWOBANK:
                        nc.vector.reciprocal(dnr, av2[D:D + 1, :])
                    else:
                        for hh in range(2):
                            hsl = slice(hh * QCH, (hh + 1) * QCH)
                            nc.vector.reciprocal(dnr[:, hsl],
                                                 av2[D:D + 1, hsl])
                av_sb = work.tile([D, 2 * QCH], F32, tag="avsb",
                                  name="av_sb")
                nc.vector.tensor_copy(out=av_sb, in_=av2[0:D, :])
                pump(4)
                for hh in range(2):
                    bcp = pj.tile([P, QCH], F32, tag="mm", name="bcps")
                    nc.tensor.matmul(bcp[0:D, :], lhsT=bc1[:, 0:D],
                                     rhs=dnr[:, hh * QCH:(hh + 1) * QCH],
                                     start=True, stop=True)
                    nc.vector.tensor_mul(
                        ao_tile[hh * D:(hh + 1) * D, :],
                        av_sb[:, hh * QCH:(hh + 1) * QCH],
                        bcp[0:D, :])

            for _rep in range(REPEAT):
                if streaming:
                    q_cur, q_nxt = {}, {}
                    # strip 1's x loads go out before strip 0's compute so
                    # its projection chains never wait on DMA
                    x_nxt = load_strip(1)
                    h0, t0, fl_cur = proj_strip_units(x0, 0, q_cur)
                    queue += h0 + t0
                    pump_all()
                    tail_cur = None
                    for ic in range(NSTRIP):
                        if ic + 2 < NSTRIP:
                            x_fut = load_strip(ic + 2)
                        else:
                            x_fut = None
                        if tail_cur:
                            # deferred pair-2/3 chains of this strip run
                            # first in this attention window
                            queue[qpos[0]:qpos[0]] = tail_cur
                        if ic + 1 < NSTRIP:
                            h2, t2, fl2 = proj_strip_units(x_nxt, ic + 1,
                                                           q_nxt)
                            queue += h2
                        else:
                            t2, fl2 = None, None
                        ao_tiles = [aopool.tile([P, QCH], F16, tag=f"aos{pp}",
                                                name=f"aos{pp}")
                                    for pp in range(PAIRS)]
                        # spread the queued units evenly over this strip's
                        # k-blocks (fractional pacing)
                        nslots = sum(
                            1 for pp in range(PAIRS) for kc in range(NKC)
                            if plan[(ic, kc)][0] != "skip")
                        todo = len(queue) - qpos[0]
                        state = {"slot": 0, "base": qpos[0]}

                        def pacer(todo=todo, nslots=max(nslots, 1),
                                  state=state):
                            state["slot"] += 1
                            target = state["base"] + (
                                state["slot"] * todo + nslots - 1) // nslots
                            pump(max(0, target - qpos[0]))
                        for pp in range(PAIRS):
                            guard = 0
                            while not fl_cur.get(pp):
                                pump(2)
                                guard += 1
                                assert guard < 10000, "tail never emitted"
                            attn_block(pp, ic, q_cur[pp], ao_tiles[pp],
                                       pacer)
                        pump_all()
                        queue += outproj_units(ic, ao_tiles,
                                               spread=(ic == NSTRIP - 1))
                        q_cur, q_nxt = q_nxt, {}
                        x_nxt = x_fut
                        tail_cur, fl_cur = t2, fl2
                    pump_all()
                    if REPEAT > 1 and _rep < REPEAT - 1:
                        x0 = load_strip(0)
                else:
                    q_all = [{} for _ in range(NSTRIP)]
                    for ic in range(NSTRIP):
                        xs = x0 if ic == 0 else load_strip(ic)
                        hh_, tt_, _fl = proj_strip_units(xs, ic, q_all[ic],
                                                         q_persistent=True)
                        queue += hh_ + tt_
                        pump_all()
                    for qc in range(NQC):
                        ao_tiles = [aopool.tile([P, QCH], F16, tag=f"aos{pp}",
                                                name=f"aos{pp}")
                                    for pp in range(PAIRS)]
                        for pp in range(PAIRS):
                            attn_block(pp, qc, qT[pp][:, qc * QCH:(qc + 1) * QCH],
                                       ao_tiles[pp], lambda: None)
                        queue += outproj_units(qc, ao_tiles)
                        pump_all()
                    if REPEAT > 1 and _rep < REPEAT - 1:
                        x0 = load_strip(0)

    nc.compile()
    return nc, masks_np


def _get_nc(is_causal, start_pos):
    key = (bool(is_causal), int(start_pos), REPEAT)
    if key not in _NC_CACHE:
        _NC_CACHE[key] = _build_nc(bool(is_causal), int(start_pos))
    return _NC_CACHE[key]


# ---------------------------------------------------------------- entry
def kernel(x_q, x_k, x_v, W_q, W_k, W_v, W_out, padding_mask, is_causal,
           start_pos):
    x_q = np.asarray(x_q, dtype=np.float32)
    x_k = np.asarray(x_k, dtype=np.float32)
    x_v = np.asarray(x_v, dtype=np.float32)
    W_q = np.asarray(W_q, dtype=np.float32)
    W_k = np.asarray(W_k, dtype=np.float32)
    W_v = np.asarray(W_v, dtype=np.float32)
    W_out = np.asarray(W_out, dtype=np.float32)
    padding_mask = np.asarray(padding_mask).astype(bool)
    is_causal = int(np.asarray(is_causal))
    start_pos = int(np.asarray(start_pos))

    nc, masks = _get_nc(is_causal, start_pos)

    cos2, sin2 = _rope_tables()
    pm = _perm_matrix()

    in_maps = []
    for c in range(NCORES):
        bi, hg = divmod(c, GROUPS)
        hs = hg * HD
        pad = padding_mask[bi]
        padb = np.where(pad.reshape(NKC, P).T, ESHIFT, ESHIFT - 30.0)
        in_maps.append({
            "xqT": np.ascontiguousarray(x_q[bi].T).astype(np.float16),
            "xkT": np.ascontiguousarray(x_k[bi].T).astype(np.float16),
            "xvT": np.ascontiguousarray(x_v[bi].T).astype(np.float16),
            "wqT": np.ascontiguousarray(W_q[hs:hs + HD].T).astype(np.float16),
            "wkT": np.ascontiguousarray(W_k[hs:hs + HD].T).astype(np.float16),
            "wvT": np.ascontiguousarray(W_v[hs:hs + HD].T).astype(np.float16),
            "woT": np.ascontiguousarray(W_out[:, hs:hs + HD].T).astype(np.float16),
            "cos": cos2,
            "sin": sin2,
            "pm": pm,
            "masks": masks,
            "padb": np.ascontiguousarray(padb, dtype=np.float32),
            "bc1": np.ones((1, P), dtype=np.float32),
        })

    res = run_bass_kernel_spmd(nc, in_maps, list(range(NCORES)))
    out = np.empty((B, N, HID), dtype=np.float32)
    for bi in range(B):
        out[bi] = res.results[GROUPS * bi]["y"]
        for g in range(1, GROUPS):
            out[bi] += res.results[GROUPS * bi + g]["y"]
    return out
